# revision 3
# baseline (speedup 1.0000x reference)
"""CreditRiskGNN (2-layer GCN) Trainium2 kernel, 8 NeuronCores.

Sharding (per spec hint): nodes sharded across the 8 cores; edges partitioned
by destination node so scatter-adds are core-local; the per-shard node
features are all-gathered between layers.

Math: GCNConv(x, W, b)[d] = dinv[d] * (sum_{e: dst=d} h'[src_e] + h'[d]) + b
where h' = dinv (.) (x @ W) and dinv = rsqrt(indegree + 1) (self-loops).
Layer 2 uses (A @ R') @ W2 == A @ (R' @ W2) associativity so both layers share
one aggregation structure.

Device pipeline per core (one NEFF, SPMD on all 8 cores; per-core data only):
  A) h'_shard = dinv (.) (x_shard @ W1)        (PE matmul + DVE scale)
  B) AllGather h' -> full table [N, 64] in DRAM
  C) layer-1 aggregation per 128-dst tile: dma_gather of h'[src] rows
     (SWDGE ucode, 4 queues round-robin), one-hot dst-selection built on DVE
     (is_equal vs iota), PE matmuls accumulate into PSUM; fused epilogue
     R' = dinv (.) relu(dinv (.) (agg + self) + b1)
  D) AllGather R'
  E) layer-2 aggregation over the same edges; y = sigmoid(dinv*(agg2@W2)+b2)

Host does graph preprocessing only (CSR sharding, degree counts, gather-index
layout) and the final shard concat.
"""

import contextlib
import ctypes
import math
import os
import sys
import types

import numpy as np

N_CORES = 8
P = 128
D_HID = 64
WIN = 32768                # int16 index window for dma_gather
MAX_IDX_PER_GATHER = 1024  # HW descriptor-ring limit (2048 hangs the queue)

LAST_RESULTS = None  # BassKernelResults of the last run (for test harnesses)


# ---------------------------------------------------------------------------
# axon NTFF profile hook shim (only needed when BASS_TRACE=1 under axon)
def _install_axon_profile_shim():
    if "antenv.axon_hooks" in sys.modules:
        return
    try:
        so_path = "/opt/axon/libaxon_pjrt.so"
        if not os.path.exists(so_path):
            return
        lib = ctypes.CDLL(so_path)
        if not hasattr(lib, "axon_start_nrt_profile"):
            return
        lib.axon_start_nrt_profile.argtypes = [
            ctypes.POINTER(ctypes.c_int64),
            ctypes.c_size_t,
        ]
        lib.axon_start_nrt_profile.restype = ctypes.c_int64
        lib.axon_stop_nrt_profile.argtypes = [ctypes.c_char_p]
        lib.axon_stop_nrt_profile.restype = ctypes.c_int64

        @contextlib.contextmanager
        def _hook(output_dir, device_ids):
            import jax

            jax.devices()
            if device_ids:
                ids = (ctypes.c_int64 * len(device_ids))(*device_ids)
                rc = lib.axon_start_nrt_profile(ids, len(device_ids))
            else:
                rc = lib.axon_start_nrt_profile(None, 0)
            if rc != 0:
                raise RuntimeError(f"axon_start_nrt_profile rc={rc}")
            try:
                yield
            finally:
                n = lib.axon_stop_nrt_profile(str(output_dir).encode())
                if n < 0:
                    raise RuntimeError(f"axon_stop_nrt_profile rc={n}")

        mod = types.ModuleType("antenv.axon_hooks")
        _state = {"hook": _hook}
        mod.set_axon_ntff_profile_hook = lambda h: _state.__setitem__("hook", h)
        mod.get_axon_ntff_profile_hook = lambda: _state["hook"]
        sys.modules["antenv.axon_hooks"] = mod
        import antenv

        antenv.axon_hooks = mod
    except Exception:
        pass


# ---------------------------------------------------------------------------
# Host-side graph preprocessing


def _wrap_idx_block(idxs_i16: np.ndarray) -> np.ndarray:
    """[n] int16 -> [128, n//16] in the SWDGE ucode layout: idx i at
    [i%16, i//16], replicated across the 8 groups of 16 partitions."""
    n = idxs_i16.shape[0]
    block = np.zeros((16, n // 16), dtype=np.int16)
    i = np.arange(n)
    block[i % 16, i // 16] = idxs_i16
    return np.tile(block, (8, 1))


def _build_plan(src, dst, n_nodes, n_cores):
    """Partition edges by destination shard; group per (dst-tile, src-window);
    pad each group to the max count across cores (rounded to 128) so the
    program shape is identical on every core."""
    sh = n_nodes // n_cores
    n_tiles = math.ceil(sh / P)
    n_win = math.ceil(n_nodes / WIN)

    core_of = dst // sh
    counts = np.zeros((n_cores, n_tiles, n_win), dtype=np.int64)
    per_core_sorted = []
    for c in range(n_cores):
        m = core_of == c
        s_c = src[m].astype(np.int64)
        d_c = (dst[m] - c * sh).astype(np.int64)
        tile_id = d_c // P
        win = s_c // WIN
        order = np.lexsort((s_c, win, tile_id))
        s_c, d_c = s_c[order], d_c[order]
        key = (d_c // P) * n_win + (s_c // WIN)
        allkeys = np.arange(n_tiles * n_win)
        starts = np.searchsorted(key, allkeys, side="left").reshape(n_tiles, n_win)
        ends = np.searchsorted(key, allkeys, side="right").reshape(n_tiles, n_win)
        counts[c] = ends - starts
        per_core_sorted.append((s_c, d_c, starts))

    padded = counts.max(axis=0)
    padded = np.where(padded > 0, ((padded + P - 1) // P) * P, 0).astype(np.int64)

    groups = []  # (t, w, ni, idx_off16, chunk_off)
    tile_nch = np.zeros(n_tiles, dtype=np.int64)
    tile_chunk_off = np.zeros(n_tiles, dtype=np.int64)
    off16 = 0
    chunk_off = 0
    for t in range(n_tiles):
        tile_chunk_off[t] = chunk_off
        for w in range(n_win):
            ni = int(padded[t, w])
            if ni == 0:
                continue
            groups.append((t, w, ni, off16, chunk_off))
            off16 += ni // 16
            chunk_off += ni // P
        tile_nch[t] = chunk_off - tile_chunk_off[t]

    meta = dict(
        n_nodes=n_nodes,
        sh=sh,
        n_tiles=n_tiles,
        n_win=n_win,
        groups=groups,
        tile_nch=tile_nch,
        tile_chunk_off=tile_chunk_off,
        total_idx=off16 * 16,
        total_chunks=int(chunk_off),
    )

    per_core = []
    for c in range(n_cores):
        s_c, d_c, starts = per_core_sorted[c]
        idx_arr = np.zeros((P, meta["total_idx"] // 16), dtype=np.int16)
        dst_arr = np.full((P, meta["total_chunks"]), -1.0, dtype=np.float32)
        for (t, w, ni, o16, ch_off) in groups:
            n_real = int(counts[c, t, w])
            st = int(starts[t, w])
            rel = np.zeros(ni, dtype=np.int16)
            if n_real > 0:
                rel[:n_real] = (s_c[st : st + n_real] - w * WIN).astype(np.int16)
            idx_arr[:, o16 : o16 + ni // 16] = _wrap_idx_block(rel)
            if n_real > 0:
                i = np.arange(n_real)
                dst_arr[i % P, ch_off + i // P] = (d_c[st : st + n_real] % P).astype(
                    np.float32
                )
        per_core.append((idx_arr, dst_arr))
    return meta, per_core


# ---------------------------------------------------------------------------
# Device program


def _build_program(meta):
    import concourse.bacc as bacc
    import concourse.mybir as mybir
    import concourse.tile as tile

    n_nodes = meta["n_nodes"]
    sh = meta["sh"]
    n_tiles = meta["n_tiles"]
    groups = meta["groups"]
    tile_nch = meta["tile_nch"]
    tile_chunk_off = meta["tile_chunk_off"]
    total_idx = meta["total_idx"]
    total_chunks = meta["total_chunks"]

    f32 = mybir.dt.float32
    nc = bacc.Bacc("TRN2", target_bir_lowering=False, debug=False, num_swdge_queues=4)

    xT = nc.dram_tensor("xT", [P, sh], f32, kind="ExternalInput")
    w1 = nc.dram_tensor("w1", [P, D_HID], f32, kind="ExternalInput")
    b1bc = nc.dram_tensor("b1bc", [P, D_HID], f32, kind="ExternalInput")
    w2bc = nc.dram_tensor("w2bc", [P, D_HID], f32, kind="ExternalInput")
    dinv_sh = nc.dram_tensor("dinv_sh", [P, n_tiles], f32, kind="ExternalInput")
    iota = nc.dram_tensor("iota", [P, P], f32, kind="ExternalInput")
    idx16 = nc.dram_tensor(
        "idx16", [P, total_idx // 16], mybir.dt.int16, kind="ExternalInput"
    )
    dstloc = nc.dram_tensor("dstloc", [P, total_chunks], f32, kind="ExternalInput")
    b2col = nc.dram_tensor("b2col", [P, 1], f32, kind="ExternalInput")
    y_out = nc.dram_tensor("y", [sh, 1], f32, kind="ExternalOutput")

    h_sh = nc.dram_tensor("h_sh", [sh, D_HID], f32, kind="Internal")
    h_full = nc.dram_tensor(
        "h_full", [n_nodes, D_HID], f32, kind="Internal", addr_space="Shared"
    )
    r_sh = nc.dram_tensor("r_sh", [sh, D_HID], f32, kind="Internal")
    r_full = nc.dram_tensor(
        "r_full", [n_nodes, D_HID], f32, kind="Internal", addr_space="Shared"
    )

    rg = [list(range(N_CORES))]

    with tile.TileContext(nc) as tc:
        with (
            tc.tile_pool(name="const", bufs=1) as cpool,
            tc.tile_pool(name="sbuf", bufs=1) as pool,
            tc.tile_pool(name="psum", bufs=1, space="PSUM") as psum_pool,
        ):
            w1_t = cpool.tile([P, D_HID], f32)
            nc.sync.dma_start(w1_t[:], w1[:])
            b1_t = cpool.tile([P, D_HID], f32)
            nc.sync.dma_start(b1_t[:], b1bc[:])
            w2_t = cpool.tile([P, D_HID], f32)
            nc.sync.dma_start(w2_t[:], w2bc[:])
            dinv_t = cpool.tile([P, n_tiles], f32)
            nc.sync.dma_start(dinv_t[:], dinv_sh[:])
            iota_t = cpool.tile([P, P], f32)
            nc.sync.dma_start(iota_t[:], iota[:])
            idx_t = cpool.tile([P, total_idx // 16], mybir.dt.int16)
            nc.sync.dma_start(idx_t[:], idx16[:])
            dl_t = cpool.tile([P, total_chunks], f32)
            nc.sync.dma_start(dl_t[:], dstloc[:])
            b2_t = cpool.tile([P, 1], f32)
            nc.sync.dma_start(b2_t[:], b2col[:])

            # ---- phase A: h' = dinv (.) (x @ W1) -> h_sh
            for t in range(n_tiles):
                pt = min(P, sh - t * P)
                xt = pool.tile([P, P], f32, tag="xt", bufs=3)
                nc.sync.dma_start(xt[:, :pt], xT[:, t * P : t * P + pt])
                ph = psum_pool.tile([P, D_HID], f32, tag="ph", bufs=2, space="PSUM")
                nc.tensor.matmul(
                    ph[:pt, :], lhsT=xt[:, :pt], rhs=w1_t[:], start=True, stop=True
                )
                hs = pool.tile([P, D_HID], f32, tag="hs", bufs=3)
                nc.vector.tensor_scalar_mul(
                    hs[:pt, :], ph[:pt, :], dinv_t[:pt, t : t + 1]
                )
                nc.sync.dma_start(h_sh[t * P : t * P + pt, :], hs[:pt, :])

            # ---- phase B: AllGather h'
            nc.gpsimd.collective_compute(
                "AllGather",
                mybir.AluOpType.bypass,
                replica_groups=rg,
                ins=[h_sh[:]],
                outs=[h_full[:]],
            )

            qn_state = [0]

            def agg_layer(table, self_src, layer):
                for t in range(n_tiles):
                    pt = min(P, sh - t * P)
                    nch = int(tile_nch[t])
                    ch0 = int(tile_chunk_off[t])
                    if nch > 0:
                        gbuf = pool.tile(
                            [P, nch, D_HID], f32, tag=f"g{layer}", bufs=3
                        )
                        col = 0
                        for (gt, w, ni, o16, ch_off) in groups:
                            if gt != t:
                                continue
                            base = w * WIN
                            span = min(WIN, n_nodes - base)
                            done = 0
                            while done < ni:
                                take = min(MAX_IDX_PER_GATHER, ni - done)
                                nc.gpsimd.dma_gather(
                                    gbuf[:, col : col + take // P, :],
                                    table[base : base + span, :],
                                    idx_t[
                                        :,
                                        o16 + done // 16 : o16 + (done + take) // 16,
                                    ],
                                    take,
                                    take,
                                    D_HID,
                                    queue_num=qn_state[0] % 4,
                                )
                                qn_state[0] += 1
                                done += take
                                col += take // P
                    st = pool.tile([P, D_HID], f32, tag=f"st{layer}", bufs=3)
                    if pt < P:
                        nc.vector.memset(st[:], 0.0)
                    nc.sync.dma_start(st[:pt, :], self_src[t * P : t * P + pt, :])
                    if nch > 0:
                        pa = psum_pool.tile(
                            [P, D_HID], f32, tag=f"pa{layer}", bufs=2, space="PSUM"
                        )
                        for ch in range(nch):
                            oh = pool.tile([P, P], f32, tag=f"oh{layer}", bufs=4)
                            nc.vector.tensor_scalar(
                                oh[:],
                                iota_t[:],
                                dl_t[:, ch0 + ch : ch0 + ch + 1],
                                None,
                                op0=mybir.AluOpType.is_equal,
                            )
                            nc.tensor.matmul(
                                pa[:],
                                lhsT=oh[:],
                                rhs=gbuf[:, ch, :],
                                start=(ch == 0),
                                stop=(ch == nch - 1),
                            )
                    dv = dinv_t[:pt, t : t + 1]
                    if layer == 1:
                        t1 = pool.tile([P, D_HID], f32, tag="t1", bufs=3)
                        if nch > 0:
                            nc.vector.tensor_add(t1[:pt, :], pa[:pt, :], st[:pt, :])
                        else:
                            nc.vector.tensor_copy(out=t1[:pt, :], in_=st[:pt, :])
                        t2 = pool.tile([P, D_HID], f32, tag="t2", bufs=3)
                        nc.vector.tensor_scalar_mul(t2[:pt, :], t1[:pt, :], dv)
                        t3 = pool.tile([P, D_HID], f32, tag="t3", bufs=3)
                        nc.vector.tensor_add(t3[:pt, :], t2[:pt, :], b1_t[:pt, :])
                        rr = pool.tile([P, D_HID], f32, tag="rr", bufs=3)
                        nc.scalar.activation(
                            rr[:pt, :], t3[:pt, :], mybir.ActivationFunctionType.Relu
                        )
                        rp = pool.tile([P, D_HID], f32, tag="rp", bufs=3)
                        nc.vector.tensor_scalar_mul(rp[:pt, :], rr[:pt, :], dv)
                        nc.scalar.dma_start(r_sh[t * P : t * P + pt, :], rp[:pt, :])
                    else:
                        u1 = pool.tile([P, D_HID], f32, tag="u1", bufs=3)
                        if nch > 0:
                            nc.vector.tensor_add(u1[:pt, :], pa[:pt, :], st[:pt, :])
                        else:
                            nc.vector.tensor_copy(out=u1[:pt, :], in_=st[:pt, :])
                        u2 = pool.tile([P, D_HID], f32, tag="u2", bufs=3)
                        nc.vector.tensor_mul(u2[:pt, :], u1[:pt, :], w2_t[:pt, :])
                        yv = pool.tile([P, 1], f32, tag="yv", bufs=3)
                        nc.vector.tensor_reduce(
                            yv[:pt, :],
                            u2[:pt, :],
                            axis=mybir.AxisListType.X,
                            op=mybir.AluOpType.add,
                        )
                        ov = pool.tile([P, 1], f32, tag="ov", bufs=3)
                        nc.scalar.activation(
                            ov[:pt, :],
                            yv[:pt, :],
                            mybir.ActivationFunctionType.Sigmoid,
                            bias=b2_t[:pt, :],
                            scale=dv,
                        )
                        nc.scalar.dma_start(y_out[t * P : t * P + pt, :], ov[:pt, :])

            # ---- phase C: layer 1 (table = h_full, self rows = local h_sh)
            agg_layer(h_full, h_sh, layer=1)

            # ---- phase D: AllGather R'
            nc.gpsimd.collective_compute(
                "AllGather",
                mybir.AluOpType.bypass,
                replica_groups=rg,
                ins=[r_sh[:]],
                outs=[r_full[:]],
            )

            # ---- phase E: layer 2
            agg_layer(r_full, r_sh, layer=2)

    nc.compile()
    return nc


# ---------------------------------------------------------------------------


def kernel(**inputs) -> np.ndarray:
    global LAST_RESULTS
    x = np.asarray(inputs["x"], dtype=np.float32)
    edge_index = np.asarray(inputs["edge_index"])
    w1_in = np.asarray(inputs["W1"], dtype=np.float32)
    b1_in = np.asarray(inputs["b1"], dtype=np.float32)
    w2_in = np.asarray(inputs["W2"], dtype=np.float32)
    b2_in = np.asarray(inputs["b2"], dtype=np.float32)

    n_nodes = x.shape[0]
    src = edge_index[0].astype(np.int64)
    dst = edge_index[1].astype(np.int64)

    deg = np.bincount(dst, minlength=n_nodes).astype(np.float64) + 1.0
    dinv = (1.0 / np.sqrt(deg)).astype(np.float32)

    meta, per_core = _build_plan(src, dst, n_nodes, N_CORES)
    sh = meta["sh"]
    n_tiles = meta["n_tiles"]

    nc = _build_program(meta)

    iota_arr = np.broadcast_to(np.arange(P, dtype=np.float32), (P, P)).copy()
    b1bc = np.broadcast_to(b1_in.reshape(1, D_HID), (P, D_HID)).copy()
    w2bc = np.broadcast_to(w2_in.reshape(1, D_HID), (P, D_HID)).copy()

    in_maps = []
    for c in range(N_CORES):
        idx_arr, dst_arr = per_core[c]
        xs = x[c * sh : (c + 1) * sh]  # [sh, 128]
        xT = np.ascontiguousarray(xs.T)  # [128, sh]
        dv = np.zeros((P, n_tiles), dtype=np.float32)
        dsl = dinv[c * sh : (c + 1) * sh]
        for t in range(n_tiles):
            pt = min(P, sh - t * P)
            dv[:pt, t] = dsl[t * P : t * P + pt]
        in_maps.append(
            {
                "xT": xT,
                "w1": w1_in,
                "b1bc": b1bc,
                "w2bc": w2bc,
                "dinv_sh": dv,
                "iota": iota_arr,
                "idx16": idx_arr,
                "dstloc": dst_arr,
                "b2col": np.full((P, 1), float(b2_in.reshape(-1)[0]), dtype=np.float32),
            }
        )

    from concourse import bass_utils

    if os.environ.get("BASS_TRACE"):
        _install_axon_profile_shim()

    res = bass_utils.run_bass_kernel_spmd(
        nc,
        in_maps,
        core_ids=list(range(N_CORES)),
        trace=bool(os.environ.get("BASS_TRACE")),
        trace_cores=[0] if os.environ.get("BASS_TRACE") else None,
    )
    LAST_RESULTS = res
    out = np.concatenate([res.results[c]["y"] for c in range(N_CORES)], axis=0)
    return out.astype(np.float32)


# revision 5
# speedup vs baseline: 1.0211x; 1.0211x over previous
"""CreditRiskGNN (2-layer GCN) Trainium2 kernel, 8 NeuronCores.

Sharding (per spec hint): nodes sharded across the 8 cores; edges partitioned
by destination node so scatter-adds are core-local; the per-shard node
features are all-gathered between layers.

Math: GCNConv(x, W, b)[d] = dinv[d] * (sum_{e: dst=d} h'[src_e] + h'[d]) + b
where h' = dinv (.) (x @ W) and dinv = rsqrt(indegree + 1) (self-loops).
Layer 2 uses (A @ R') @ W2 == A @ (R' @ W2) associativity so both layers share
one aggregation structure.

Device pipeline per core (one NEFF, SPMD on all 8 cores; per-core data only):
  A) h'_shard = dinv (.) (x_shard @ W1)        (PE matmul + DVE scale)
  B) AllGather h' -> full table [N, 64] in DRAM
  C) layer-1 aggregation per 128-dst tile: dma_gather of h'[src] rows
     (SWDGE ucode, 4 queues round-robin), one-hot dst-selection built on DVE
     (is_equal vs iota), PE matmuls accumulate into PSUM; fused epilogue
     R' = dinv (.) relu(dinv (.) (agg + self) + b1)
  D) AllGather R'
  E) layer-2 aggregation over the same edges; y = sigmoid(dinv*(agg2@W2)+b2)

Host does graph preprocessing only (CSR sharding, degree counts, gather-index
layout) and the final shard concat.
"""

import contextlib
import ctypes
import math
import os
import sys
import types

import numpy as np

N_CORES = 8
P = 128
D_HID = 64
WIN = 32768                # int16 index window for dma_gather
MAX_IDX_PER_GATHER = 1024  # HW descriptor-ring limit (2048 hangs the queue)

LAST_RESULTS = None  # BassKernelResults of the last run (for test harnesses)


# ---------------------------------------------------------------------------
# axon NTFF profile hook shim (only needed when BASS_TRACE=1 under axon)
def _install_axon_profile_shim():
    if "antenv.axon_hooks" in sys.modules:
        return
    try:
        so_path = "/opt/axon/libaxon_pjrt.so"
        if not os.path.exists(so_path):
            return
        lib = ctypes.CDLL(so_path)
        if not hasattr(lib, "axon_start_nrt_profile"):
            return
        lib.axon_start_nrt_profile.argtypes = [
            ctypes.POINTER(ctypes.c_int64),
            ctypes.c_size_t,
        ]
        lib.axon_start_nrt_profile.restype = ctypes.c_int64
        lib.axon_stop_nrt_profile.argtypes = [ctypes.c_char_p]
        lib.axon_stop_nrt_profile.restype = ctypes.c_int64

        @contextlib.contextmanager
        def _hook(output_dir, device_ids):
            import jax

            jax.devices()
            if device_ids:
                ids = (ctypes.c_int64 * len(device_ids))(*device_ids)
                rc = lib.axon_start_nrt_profile(ids, len(device_ids))
            else:
                rc = lib.axon_start_nrt_profile(None, 0)
            if rc != 0:
                raise RuntimeError(f"axon_start_nrt_profile rc={rc}")
            try:
                yield
            finally:
                n = lib.axon_stop_nrt_profile(str(output_dir).encode())
                if n < 0:
                    raise RuntimeError(f"axon_stop_nrt_profile rc={n}")

        mod = types.ModuleType("antenv.axon_hooks")
        _state = {"hook": _hook}
        mod.set_axon_ntff_profile_hook = lambda h: _state.__setitem__("hook", h)
        mod.get_axon_ntff_profile_hook = lambda: _state["hook"]
        sys.modules["antenv.axon_hooks"] = mod
        import antenv

        antenv.axon_hooks = mod
    except Exception:
        pass


# ---------------------------------------------------------------------------
# Host-side graph preprocessing


def _wrap_idx_block(idxs_i16: np.ndarray) -> np.ndarray:
    """[n] int16 -> [128, n//16] in the SWDGE ucode layout: idx i at
    [i%16, i//16], replicated across the 8 groups of 16 partitions."""
    n = idxs_i16.shape[0]
    block = np.zeros((16, n // 16), dtype=np.int16)
    i = np.arange(n)
    block[i % 16, i // 16] = idxs_i16
    return np.tile(block, (8, 1))


def _build_plan(src, dst, n_nodes, n_cores):
    """Partition edges by destination shard; group per (dst-tile, src-window);
    pad each group to the max count across cores (rounded to 128) so the
    program shape is identical on every core."""
    sh = n_nodes // n_cores
    n_tiles = math.ceil(sh / P)
    n_win = math.ceil(n_nodes / WIN)

    core_of = dst // sh
    counts = np.zeros((n_cores, n_tiles, n_win), dtype=np.int64)
    per_core_sorted = []
    for c in range(n_cores):
        m = core_of == c
        s_c = src[m].astype(np.int64)
        d_c = (dst[m] - c * sh).astype(np.int64)
        tile_id = d_c // P
        win = s_c // WIN
        order = np.lexsort((s_c, win, tile_id))
        s_c, d_c = s_c[order], d_c[order]
        key = (d_c // P) * n_win + (s_c // WIN)
        allkeys = np.arange(n_tiles * n_win)
        starts = np.searchsorted(key, allkeys, side="left").reshape(n_tiles, n_win)
        ends = np.searchsorted(key, allkeys, side="right").reshape(n_tiles, n_win)
        counts[c] = ends - starts
        per_core_sorted.append((s_c, d_c, starts))

    padded = counts.max(axis=0)
    padded = np.where(padded > 0, ((padded + P - 1) // P) * P, 0).astype(np.int64)

    groups = []  # (t, w, ni, idx_off16, chunk_off)
    tile_nch = np.zeros(n_tiles, dtype=np.int64)
    tile_chunk_off = np.zeros(n_tiles, dtype=np.int64)
    off16 = 0
    chunk_off = 0
    for t in range(n_tiles):
        tile_chunk_off[t] = chunk_off
        for w in range(n_win):
            ni = int(padded[t, w])
            if ni == 0:
                continue
            groups.append((t, w, ni, off16, chunk_off))
            off16 += ni // 16
            chunk_off += ni // P
        tile_nch[t] = chunk_off - tile_chunk_off[t]

    meta = dict(
        n_nodes=n_nodes,
        sh=sh,
        n_tiles=n_tiles,
        n_win=n_win,
        groups=groups,
        tile_nch=tile_nch,
        tile_chunk_off=tile_chunk_off,
        total_idx=off16 * 16,
        total_chunks=int(chunk_off),
    )

    per_core = []
    for c in range(n_cores):
        s_c, d_c, starts = per_core_sorted[c]
        idx_arr = np.zeros((P, meta["total_idx"] // 16), dtype=np.int16)
        dst_arr = np.full((P, meta["total_chunks"]), -1.0, dtype=np.float32)
        for (t, w, ni, o16, ch_off) in groups:
            n_real = int(counts[c, t, w])
            st = int(starts[t, w])
            rel = np.zeros(ni, dtype=np.int16)
            if n_real > 0:
                rel[:n_real] = (s_c[st : st + n_real] - w * WIN).astype(np.int16)
            idx_arr[:, o16 : o16 + ni // 16] = _wrap_idx_block(rel)
            if n_real > 0:
                i = np.arange(n_real)
                dst_arr[i % P, ch_off + i // P] = (d_c[st : st + n_real] % P).astype(
                    np.float32
                )
        per_core.append((idx_arr, dst_arr))
    return meta, per_core


# ---------------------------------------------------------------------------
# Device program


def _build_program(meta):
    import concourse.bacc as bacc
    import concourse.mybir as mybir
    import concourse.tile as tile

    n_nodes = meta["n_nodes"]
    sh = meta["sh"]
    n_tiles = meta["n_tiles"]
    groups = meta["groups"]
    tile_nch = meta["tile_nch"]
    tile_chunk_off = meta["tile_chunk_off"]
    total_idx = meta["total_idx"]
    total_chunks = meta["total_chunks"]

    f32 = mybir.dt.float32
    nc = bacc.Bacc("TRN2", target_bir_lowering=False, debug=False, num_swdge_queues=4)

    xT = nc.dram_tensor("xT", [P, sh], f32, kind="ExternalInput")
    w1 = nc.dram_tensor("w1", [P, D_HID], f32, kind="ExternalInput")
    b1bc = nc.dram_tensor("b1bc", [P, D_HID], f32, kind="ExternalInput")
    w2bc = nc.dram_tensor("w2bc", [P, D_HID], f32, kind="ExternalInput")
    dinv_sh = nc.dram_tensor("dinv_sh", [P, n_tiles], f32, kind="ExternalInput")
    iota = nc.dram_tensor("iota", [P, P], f32, kind="ExternalInput")
    iota4 = nc.dram_tensor("iota4", [P, 4, P], f32, kind="ExternalInput")
    idx16 = nc.dram_tensor(
        "idx16", [P, total_idx // 16], mybir.dt.int16, kind="ExternalInput"
    )
    dstloc = nc.dram_tensor("dstloc", [P, total_chunks], f32, kind="ExternalInput")
    b2col = nc.dram_tensor("b2col", [P, 1], f32, kind="ExternalInput")
    y_out = nc.dram_tensor("y", [sh, 1], f32, kind="ExternalOutput")

    h_sh = nc.dram_tensor("h_sh", [sh, D_HID], f32, kind="Internal")
    h_full = nc.dram_tensor(
        "h_full", [n_nodes, D_HID], f32, kind="Internal", addr_space="Shared"
    )
    r_sh = nc.dram_tensor("r_sh", [sh, D_HID], f32, kind="Internal")
    r_full = nc.dram_tensor(
        "r_full", [n_nodes, D_HID], f32, kind="Internal", addr_space="Shared"
    )

    rg = [list(range(N_CORES))]

    with tile.TileContext(nc) as tc:
        with (
            tc.tile_pool(name="const", bufs=1) as cpool,
            tc.tile_pool(name="sbuf", bufs=1) as pool,
            tc.tile_pool(name="psum", bufs=1, space="PSUM") as psum_pool,
        ):
            w1_t = cpool.tile([P, D_HID], f32)
            nc.sync.dma_start(w1_t[:], w1[:])
            b1_t = cpool.tile([P, D_HID], f32)
            nc.sync.dma_start(b1_t[:], b1bc[:])
            w2_t = cpool.tile([P, D_HID], f32)
            nc.sync.dma_start(w2_t[:], w2bc[:])
            dinv_t = cpool.tile([P, n_tiles], f32)
            nc.sync.dma_start(dinv_t[:], dinv_sh[:])
            iota_t = cpool.tile([P, P], f32)
            nc.sync.dma_start(iota_t[:], iota[:])
            iota4_t = cpool.tile([P, 4, P], f32)
            nc.sync.dma_start(iota4_t[:], iota4[:])
            idx_t = cpool.tile([P, total_idx // 16], mybir.dt.int16)
            nc.sync.dma_start(idx_t[:], idx16[:])
            dl_t = cpool.tile([P, total_chunks], f32)
            nc.sync.dma_start(dl_t[:], dstloc[:])
            b2_t = cpool.tile([P, 1], f32)
            nc.sync.dma_start(b2_t[:], b2col[:])

            # ---- phase A: h' = dinv (.) (x @ W1) -> h_sh
            for t in range(n_tiles):
                pt = min(P, sh - t * P)
                xt = pool.tile([P, P], f32, tag="xt", bufs=3)
                nc.sync.dma_start(xt[:, :pt], xT[:, t * P : t * P + pt])
                ph = psum_pool.tile([P, D_HID], f32, tag="ph", bufs=2, space="PSUM")
                nc.tensor.matmul(
                    ph[:pt, :], lhsT=xt[:, :pt], rhs=w1_t[:], start=True, stop=True
                )
                hs = pool.tile([P, D_HID], f32, tag="hs", bufs=3)
                nc.vector.tensor_scalar_mul(
                    hs[:pt, :], ph[:pt, :], dinv_t[:pt, t : t + 1]
                )
                nc.sync.dma_start(h_sh[t * P : t * P + pt, :], hs[:pt, :])

            # ---- phase B: AllGather h'
            nc.gpsimd.collective_compute(
                "AllGather",
                mybir.AluOpType.bypass,
                replica_groups=rg,
                ins=[h_sh[:]],
                outs=[h_full[:]],
            )

            qn_state = [0]

            def agg_layer(table, self_src, layer):
                for t in range(n_tiles):
                    pt = min(P, sh - t * P)
                    nch = int(tile_nch[t])
                    ch0 = int(tile_chunk_off[t])
                    if nch > 0:
                        gbuf = pool.tile(
                            [P, nch, D_HID], f32, tag=f"g{layer}", bufs=3
                        )
                        col = 0
                        for (gt, w, ni, o16, ch_off) in groups:
                            if gt != t:
                                continue
                            base = w * WIN
                            span = min(WIN, n_nodes - base)
                            done = 0
                            while done < ni:
                                take = min(MAX_IDX_PER_GATHER, ni - done)
                                nc.gpsimd.dma_gather(
                                    gbuf[:, col : col + take // P, :],
                                    table[base : base + span, :],
                                    idx_t[
                                        :,
                                        o16 + done // 16 : o16 + (done + take) // 16,
                                    ],
                                    take,
                                    take,
                                    D_HID,
                                    queue_num=qn_state[0] % 4,
                                )
                                qn_state[0] += 1
                                done += take
                                col += take // P
                    st = pool.tile([P, D_HID], f32, tag=f"st{layer}", bufs=3)
                    if pt < P:
                        nc.vector.memset(st[:], 0.0)
                    nc.sync.dma_start(st[:pt, :], self_src[t * P : t * P + pt, :])
                    if nch > 0:
                        pa = psum_pool.tile(
                            [P, D_HID], f32, tag=f"pa{layer}", bufs=2, space="PSUM"
                        )
                        for cb in range(0, nch, 4):
                            b = min(4, nch - cb)
                            oh = pool.tile([P, 4, P], f32, tag=f"oh{layer}", bufs=4)
                            dls = dl_t[:, ch0 + cb : ch0 + cb + b].rearrange(
                                "p (b o) -> p b o", o=1
                            )
                            nc.vector.tensor_tensor(
                                out=oh[:, :b, :],
                                in0=dls.to_broadcast([P, b, P]),
                                in1=iota4_t[:, :b, :],
                                op=mybir.AluOpType.is_equal,
                            )
                            for k in range(b):
                                ch = cb + k
                                nc.tensor.matmul(
                                    pa[:],
                                    lhsT=oh[:, k, :],
                                    rhs=gbuf[:, ch, :],
                                    start=(ch == 0),
                                    stop=(ch == nch - 1),
                                )
                    dv = dinv_t[:pt, t : t + 1]
                    if layer == 1:
                        t1 = pool.tile([P, D_HID], f32, tag="t1", bufs=3)
                        if nch > 0:
                            nc.vector.tensor_add(t1[:pt, :], pa[:pt, :], st[:pt, :])
                        else:
                            nc.vector.tensor_copy(out=t1[:pt, :], in_=st[:pt, :])
                        t2 = pool.tile([P, D_HID], f32, tag="t2", bufs=3)
                        nc.vector.tensor_scalar_mul(t2[:pt, :], t1[:pt, :], dv)
                        t3 = pool.tile([P, D_HID], f32, tag="t3", bufs=3)
                        nc.vector.tensor_add(t3[:pt, :], t2[:pt, :], b1_t[:pt, :])
                        rr = pool.tile([P, D_HID], f32, tag="rr", bufs=3)
                        nc.scalar.activation(
                            rr[:pt, :], t3[:pt, :], mybir.ActivationFunctionType.Relu
                        )
                        rp = pool.tile([P, D_HID], f32, tag="rp", bufs=3)
                        nc.vector.tensor_scalar_mul(rp[:pt, :], rr[:pt, :], dv)
                        nc.scalar.dma_start(r_sh[t * P : t * P + pt, :], rp[:pt, :])
                    else:
                        u1 = pool.tile([P, D_HID], f32, tag="u1", bufs=3)
                        if nch > 0:
                            nc.vector.tensor_add(u1[:pt, :], pa[:pt, :], st[:pt, :])
                        else:
                            nc.vector.tensor_copy(out=u1[:pt, :], in_=st[:pt, :])
                        u2 = pool.tile([P, D_HID], f32, tag="u2", bufs=3)
                        nc.vector.tensor_mul(u2[:pt, :], u1[:pt, :], w2_t[:pt, :])
                        yv = pool.tile([P, 1], f32, tag="yv", bufs=3)
                        nc.vector.tensor_reduce(
                            yv[:pt, :],
                            u2[:pt, :],
                            axis=mybir.AxisListType.X,
                            op=mybir.AluOpType.add,
                        )
                        ov = pool.tile([P, 1], f32, tag="ov", bufs=3)
                        nc.scalar.activation(
                            ov[:pt, :],
                            yv[:pt, :],
                            mybir.ActivationFunctionType.Sigmoid,
                            bias=b2_t[:pt, :],
                            scale=dv,
                        )
                        nc.scalar.dma_start(y_out[t * P : t * P + pt, :], ov[:pt, :])

            # ---- phase C: layer 1 (table = h_full, self rows = local h_sh)
            agg_layer(h_full, h_sh, layer=1)

            # ---- phase D: AllGather R'
            nc.gpsimd.collective_compute(
                "AllGather",
                mybir.AluOpType.bypass,
                replica_groups=rg,
                ins=[r_sh[:]],
                outs=[r_full[:]],
            )

            # ---- phase E: layer 2
            agg_layer(r_full, r_sh, layer=2)

    nc.compile()
    return nc


# ---------------------------------------------------------------------------


def kernel(**inputs) -> np.ndarray:
    global LAST_RESULTS
    x = np.asarray(inputs["x"], dtype=np.float32)
    edge_index = np.asarray(inputs["edge_index"])
    w1_in = np.asarray(inputs["W1"], dtype=np.float32)
    b1_in = np.asarray(inputs["b1"], dtype=np.float32)
    w2_in = np.asarray(inputs["W2"], dtype=np.float32)
    b2_in = np.asarray(inputs["b2"], dtype=np.float32)

    n_nodes = x.shape[0]
    src = edge_index[0].astype(np.int64)
    dst = edge_index[1].astype(np.int64)

    deg = np.bincount(dst, minlength=n_nodes).astype(np.float64) + 1.0
    dinv = (1.0 / np.sqrt(deg)).astype(np.float32)

    meta, per_core = _build_plan(src, dst, n_nodes, N_CORES)
    sh = meta["sh"]
    n_tiles = meta["n_tiles"]

    nc = _build_program(meta)

    iota_arr = np.broadcast_to(np.arange(P, dtype=np.float32), (P, P)).copy()
    iota4_arr = np.broadcast_to(
        np.arange(P, dtype=np.float32), (P, 4, P)
    ).copy()
    b1bc = np.broadcast_to(b1_in.reshape(1, D_HID), (P, D_HID)).copy()
    w2bc = np.broadcast_to(w2_in.reshape(1, D_HID), (P, D_HID)).copy()

    in_maps = []
    for c in range(N_CORES):
        idx_arr, dst_arr = per_core[c]
        xs = x[c * sh : (c + 1) * sh]  # [sh, 128]
        xT = np.ascontiguousarray(xs.T)  # [128, sh]
        dv = np.zeros((P, n_tiles), dtype=np.float32)
        dsl = dinv[c * sh : (c + 1) * sh]
        for t in range(n_tiles):
            pt = min(P, sh - t * P)
            dv[:pt, t] = dsl[t * P : t * P + pt]
        in_maps.append(
            {
                "xT": xT,
                "w1": w1_in,
                "b1bc": b1bc,
                "w2bc": w2bc,
                "dinv_sh": dv,
                "iota": iota_arr,
                "iota4": iota4_arr,
                "idx16": idx_arr,
                "dstloc": dst_arr,
                "b2col": np.full((P, 1), float(b2_in.reshape(-1)[0]), dtype=np.float32),
            }
        )

    from concourse import bass_utils

    if os.environ.get("BASS_TRACE"):
        _install_axon_profile_shim()

    res = bass_utils.run_bass_kernel_spmd(
        nc,
        in_maps,
        core_ids=list(range(N_CORES)),
        trace=bool(os.environ.get("BASS_TRACE")),
        trace_cores=[0] if os.environ.get("BASS_TRACE") else None,
    )
    LAST_RESULTS = res
    out = np.concatenate([res.results[c]["y"] for c in range(N_CORES)], axis=0)
    return out.astype(np.float32)


# revision 6
# speedup vs baseline: 1.4091x; 1.3800x over previous
"""CreditRiskGNN (2-layer GCN) Trainium2 kernel, 8 NeuronCores.

Sharding (per spec hint): nodes sharded across the 8 cores; edges partitioned
by destination node so scatter-adds are core-local; the per-shard node
features are all-gathered between layers.

Math: GCNConv(x, W, b)[d] = dinv[d] * (sum_{e: dst=d} h'[src_e] + h'[d]) + b
where h' = dinv (.) (x @ W) and dinv = rsqrt(indegree + 1) (self-loops).
Layer 2 uses (A @ R') @ W2 == A @ (R' @ W2) associativity so both layers share
one aggregation structure.

Device pipeline per core (one NEFF, SPMD on all 8 cores; per-core data only):
  A) h'_shard = dinv (.) (x_shard @ W1)        (PE matmul + DVE scale)
  B) AllGather h' -> full table [N, 64] in DRAM
  C) layer-1 aggregation per 128-dst tile: dma_gather of h'[src] rows
     (SWDGE ucode, 4 queues round-robin), one-hot dst-selection built on DVE
     (is_equal vs iota), PE matmuls accumulate into PSUM; fused epilogue
     R' = dinv (.) relu(dinv (.) (agg + self) + b1)
  D) AllGather R'
  E) layer-2 aggregation over the same edges; y = sigmoid(dinv*(agg2@W2)+b2)

Host does graph preprocessing only (CSR sharding, degree counts, gather-index
layout) and the final shard concat.
"""

import contextlib
import ctypes
import math
import os
import sys
import types

import numpy as np

N_CORES = 8
P = 128
D_HID = 64
WIN = 32768                # int16 index window for dma_gather
MAX_IDX_PER_GATHER = 1024  # HW descriptor-ring limit (2048 hangs the queue)

LAST_RESULTS = None  # BassKernelResults of the last run (for test harnesses)


# ---------------------------------------------------------------------------
# axon NTFF profile hook shim (only needed when BASS_TRACE=1 under axon)
def _install_axon_profile_shim():
    if "antenv.axon_hooks" in sys.modules:
        return
    try:
        so_path = "/opt/axon/libaxon_pjrt.so"
        if not os.path.exists(so_path):
            return
        lib = ctypes.CDLL(so_path)
        if not hasattr(lib, "axon_start_nrt_profile"):
            return
        lib.axon_start_nrt_profile.argtypes = [
            ctypes.POINTER(ctypes.c_int64),
            ctypes.c_size_t,
        ]
        lib.axon_start_nrt_profile.restype = ctypes.c_int64
        lib.axon_stop_nrt_profile.argtypes = [ctypes.c_char_p]
        lib.axon_stop_nrt_profile.restype = ctypes.c_int64

        @contextlib.contextmanager
        def _hook(output_dir, device_ids):
            import jax

            jax.devices()
            if device_ids:
                ids = (ctypes.c_int64 * len(device_ids))(*device_ids)
                rc = lib.axon_start_nrt_profile(ids, len(device_ids))
            else:
                rc = lib.axon_start_nrt_profile(None, 0)
            if rc != 0:
                raise RuntimeError(f"axon_start_nrt_profile rc={rc}")
            try:
                yield
            finally:
                n = lib.axon_stop_nrt_profile(str(output_dir).encode())
                if n < 0:
                    raise RuntimeError(f"axon_stop_nrt_profile rc={n}")

        mod = types.ModuleType("antenv.axon_hooks")
        _state = {"hook": _hook}
        mod.set_axon_ntff_profile_hook = lambda h: _state.__setitem__("hook", h)
        mod.get_axon_ntff_profile_hook = lambda: _state["hook"]
        sys.modules["antenv.axon_hooks"] = mod
        import antenv

        antenv.axon_hooks = mod
    except Exception:
        pass


# ---------------------------------------------------------------------------
# Host-side graph preprocessing


def _wrap_idx_block(idxs_i16: np.ndarray) -> np.ndarray:
    """[n] int16 -> [128, n//16] in the SWDGE ucode layout: idx i at
    [i%16, i//16], replicated across the 8 groups of 16 partitions."""
    n = idxs_i16.shape[0]
    block = np.zeros((16, n // 16), dtype=np.int16)
    i = np.arange(n)
    block[i % 16, i // 16] = idxs_i16
    return np.tile(block, (8, 1))


def _build_plan(src, dst, n_nodes, n_cores):
    """Partition edges by destination shard; group per (dst-tile, src-window);
    pad each group to the max count across cores (rounded to 128) so the
    program shape is identical on every core."""
    sh = n_nodes // n_cores
    n_tiles = math.ceil(sh / P)
    n_win = math.ceil(n_nodes / WIN)

    core_of = dst // sh
    counts = np.zeros((n_cores, n_tiles, n_win), dtype=np.int64)
    per_core_sorted = []
    for c in range(n_cores):
        m = core_of == c
        s_c = src[m].astype(np.int64)
        d_c = (dst[m] - c * sh).astype(np.int64)
        tile_id = d_c // P
        win = s_c // WIN
        order = np.lexsort((s_c, win, tile_id))
        s_c, d_c = s_c[order], d_c[order]
        key = (d_c // P) * n_win + (s_c // WIN)
        allkeys = np.arange(n_tiles * n_win)
        starts = np.searchsorted(key, allkeys, side="left").reshape(n_tiles, n_win)
        ends = np.searchsorted(key, allkeys, side="right").reshape(n_tiles, n_win)
        counts[c] = ends - starts
        per_core_sorted.append((s_c, d_c, starts))

    padded = counts.max(axis=0)
    padded = np.where(padded > 0, ((padded + 15) // 16) * 16, 0).astype(np.int64)

    groups = []  # (t, w, ni, idx_off16, chunk_off)
    tile_nch = np.zeros(n_tiles, dtype=np.int64)
    tile_chunk_off = np.zeros(n_tiles, dtype=np.int64)
    off16 = 0
    chunk_off = 0
    for t in range(n_tiles):
        tile_chunk_off[t] = chunk_off
        for w in range(n_win):
            ni = int(padded[t, w])
            if ni == 0:
                continue
            groups.append((t, w, ni, off16, chunk_off))
            off16 += ni // 16
            chunk_off += (ni + P - 1) // P
        tile_nch[t] = chunk_off - tile_chunk_off[t]

    chunk_k = np.zeros(int(chunk_off), dtype=np.int64)
    for (t, w, ni, o16, ch_off) in groups:
        ncg = (ni + P - 1) // P
        for k in range(ncg):
            chunk_k[ch_off + k] = min(P, ni - k * P)

    meta = dict(
        n_nodes=n_nodes,
        sh=sh,
        n_tiles=n_tiles,
        n_win=n_win,
        groups=groups,
        tile_nch=tile_nch,
        tile_chunk_off=tile_chunk_off,
        chunk_k=chunk_k,
        total_idx=off16 * 16,
        total_chunks=int(chunk_off),
    )

    per_core = []
    for c in range(n_cores):
        s_c, d_c, starts = per_core_sorted[c]
        idx_arr = np.zeros((P, meta["total_idx"] // 16), dtype=np.int16)
        dst_arr = np.full((P, meta["total_chunks"]), -1.0, dtype=np.float32)
        for (t, w, ni, o16, ch_off) in groups:
            n_real = int(counts[c, t, w])
            st = int(starts[t, w])
            rel = np.zeros(ni, dtype=np.int16)
            if n_real > 0:
                rel[:n_real] = (s_c[st : st + n_real] - w * WIN).astype(np.int16)
            idx_arr[:, o16 : o16 + ni // 16] = _wrap_idx_block(rel)
            if n_real > 0:
                i = np.arange(n_real)
                dst_arr[i % P, ch_off + i // P] = (d_c[st : st + n_real] % P).astype(
                    np.float32
                )
        per_core.append((idx_arr, dst_arr))
    return meta, per_core


# ---------------------------------------------------------------------------
# Device program


def _build_program(meta):
    import concourse.bacc as bacc
    import concourse.mybir as mybir
    import concourse.tile as tile

    n_nodes = meta["n_nodes"]
    sh = meta["sh"]
    n_tiles = meta["n_tiles"]
    groups = meta["groups"]
    tile_nch = meta["tile_nch"]
    tile_chunk_off = meta["tile_chunk_off"]
    chunk_k = meta["chunk_k"]
    total_idx = meta["total_idx"]
    total_chunks = meta["total_chunks"]

    f32 = mybir.dt.float32
    nc = bacc.Bacc("TRN2", target_bir_lowering=False, debug=False, num_swdge_queues=4)

    xT = nc.dram_tensor("xT", [P, sh], f32, kind="ExternalInput")
    w1 = nc.dram_tensor("w1", [P, D_HID], f32, kind="ExternalInput")
    b1bc = nc.dram_tensor("b1bc", [P, D_HID], f32, kind="ExternalInput")
    w2bc = nc.dram_tensor("w2bc", [P, D_HID], f32, kind="ExternalInput")
    dinv_sh = nc.dram_tensor("dinv_sh", [P, n_tiles], f32, kind="ExternalInput")
    iota = nc.dram_tensor("iota", [P, P], f32, kind="ExternalInput")
    iota4 = nc.dram_tensor("iota4", [P, 4, P], f32, kind="ExternalInput")
    idx16 = nc.dram_tensor(
        "idx16", [P, total_idx // 16], mybir.dt.int16, kind="ExternalInput"
    )
    dstloc = nc.dram_tensor("dstloc", [P, total_chunks], f32, kind="ExternalInput")
    b2col = nc.dram_tensor("b2col", [P, 1], f32, kind="ExternalInput")
    y_out = nc.dram_tensor("y", [sh, 1], f32, kind="ExternalOutput")

    h_sh = nc.dram_tensor("h_sh", [sh, D_HID], f32, kind="Internal")
    h_full = nc.dram_tensor(
        "h_full", [n_nodes, D_HID], f32, kind="Internal", addr_space="Shared"
    )
    r_sh = nc.dram_tensor("r_sh", [sh, D_HID], f32, kind="Internal")
    r_full = nc.dram_tensor(
        "r_full", [n_nodes, D_HID], f32, kind="Internal", addr_space="Shared"
    )

    rg = [list(range(N_CORES))]

    with tile.TileContext(nc) as tc:
        with (
            tc.tile_pool(name="const", bufs=1) as cpool,
            tc.tile_pool(name="sbuf", bufs=1) as pool,
            tc.tile_pool(name="psum", bufs=1, space="PSUM") as psum_pool,
        ):
            w1_t = cpool.tile([P, D_HID], f32)
            nc.sync.dma_start(w1_t[:], w1[:])
            b1_t = cpool.tile([P, D_HID], f32)
            nc.sync.dma_start(b1_t[:], b1bc[:])
            w2_t = cpool.tile([P, D_HID], f32)
            nc.sync.dma_start(w2_t[:], w2bc[:])
            dinv_t = cpool.tile([P, n_tiles], f32)
            nc.sync.dma_start(dinv_t[:], dinv_sh[:])
            iota_t = cpool.tile([P, P], f32)
            nc.sync.dma_start(iota_t[:], iota[:])
            iota4_t = cpool.tile([P, 4, P], f32)
            nc.sync.dma_start(iota4_t[:], iota4[:])
            idx_t = cpool.tile([P, total_idx // 16], mybir.dt.int16)
            nc.sync.dma_start(idx_t[:], idx16[:])
            dl_t = cpool.tile([P, total_chunks], f32)
            nc.sync.dma_start(dl_t[:], dstloc[:])
            b2_t = cpool.tile([P, 1], f32)
            nc.sync.dma_start(b2_t[:], b2col[:])

            # ---- phase A: h' = dinv (.) (x @ W1) -> h_sh
            for t in range(n_tiles):
                pt = min(P, sh - t * P)
                xt = pool.tile([P, P], f32, tag="xt", bufs=3)
                nc.sync.dma_start(xt[:, :pt], xT[:, t * P : t * P + pt])
                ph = psum_pool.tile([P, D_HID], f32, tag="ph", bufs=2, space="PSUM")
                nc.tensor.matmul(
                    ph[:pt, :], lhsT=xt[:, :pt], rhs=w1_t[:], start=True, stop=True
                )
                hs = pool.tile([P, D_HID], f32, tag="hs", bufs=3)
                nc.vector.tensor_scalar_mul(
                    hs[:pt, :], ph[:pt, :], dinv_t[:pt, t : t + 1]
                )
                nc.sync.dma_start(h_sh[t * P : t * P + pt, :], hs[:pt, :])

            # ---- phase B: AllGather h'
            nc.gpsimd.collective_compute(
                "AllGather",
                mybir.AluOpType.bypass,
                replica_groups=rg,
                ins=[h_sh[:]],
                outs=[h_full[:]],
            )

            qn_state = [0]

            def agg_layer(table, self_src, layer):
                for t in range(n_tiles):
                    pt = min(P, sh - t * P)
                    nch = int(tile_nch[t])
                    ch0 = int(tile_chunk_off[t])
                    if nch > 0:
                        gbuf = pool.tile(
                            [P, nch, D_HID], f32, tag=f"g{layer}", bufs=3
                        )
                        col = 0
                        for (gt, w, ni, o16, ch_off) in groups:
                            if gt != t:
                                continue
                            base = w * WIN
                            span = min(WIN, n_nodes - base)
                            done = 0
                            while done < ni:
                                take = min(MAX_IDX_PER_GATHER, ni - done)
                                ncg = (take + P - 1) // P
                                nc.gpsimd.dma_gather(
                                    gbuf[:, col : col + ncg, :],
                                    table[base : base + span, :],
                                    idx_t[
                                        :,
                                        o16 + done // 16 : o16 + (done + take) // 16,
                                    ],
                                    take,
                                    take,
                                    D_HID,
                                    queue_num=qn_state[0] % 4,
                                )
                                qn_state[0] += 1
                                done += take
                                col += ncg
                    st = pool.tile([P, D_HID], f32, tag=f"st{layer}", bufs=3)
                    if pt < P:
                        nc.vector.memset(st[:], 0.0)
                    nc.sync.dma_start(st[:pt, :], self_src[t * P : t * P + pt, :])
                    if nch > 0:
                        pa = psum_pool.tile(
                            [P, D_HID], f32, tag=f"pa{layer}", bufs=2, space="PSUM"
                        )
                        for cb in range(0, nch, 4):
                            b = min(4, nch - cb)
                            oh = pool.tile([P, 4, P], f32, tag=f"oh{layer}", bufs=4)
                            dls = dl_t[:, ch0 + cb : ch0 + cb + b].rearrange(
                                "p (b o) -> p b o", o=1
                            )
                            nc.vector.tensor_tensor(
                                out=oh[:, :b, :],
                                in0=dls.to_broadcast([P, b, P]),
                                in1=iota4_t[:, :b, :],
                                op=mybir.AluOpType.is_equal,
                            )
                            for k in range(b):
                                ch = cb + k
                                kk = int(chunk_k[ch0 + ch])
                                nc.tensor.matmul(
                                    pa[:],
                                    lhsT=oh[:kk, k, :],
                                    rhs=gbuf[:kk, ch, :],
                                    start=(ch == 0),
                                    stop=(ch == nch - 1),
                                )
                    dv = dinv_t[:pt, t : t + 1]
                    if layer == 1:
                        t1 = pool.tile([P, D_HID], f32, tag="t1", bufs=3)
                        if nch > 0:
                            nc.vector.tensor_add(t1[:pt, :], pa[:pt, :], st[:pt, :])
                        else:
                            nc.vector.tensor_copy(out=t1[:pt, :], in_=st[:pt, :])
                        t2 = pool.tile([P, D_HID], f32, tag="t2", bufs=3)
                        nc.vector.tensor_scalar_mul(t2[:pt, :], t1[:pt, :], dv)
                        t3 = pool.tile([P, D_HID], f32, tag="t3", bufs=3)
                        nc.vector.tensor_add(t3[:pt, :], t2[:pt, :], b1_t[:pt, :])
                        rr = pool.tile([P, D_HID], f32, tag="rr", bufs=3)
                        nc.scalar.activation(
                            rr[:pt, :], t3[:pt, :], mybir.ActivationFunctionType.Relu
                        )
                        rp = pool.tile([P, D_HID], f32, tag="rp", bufs=3)
                        nc.vector.tensor_scalar_mul(rp[:pt, :], rr[:pt, :], dv)
                        nc.scalar.dma_start(r_sh[t * P : t * P + pt, :], rp[:pt, :])
                    else:
                        u1 = pool.tile([P, D_HID], f32, tag="u1", bufs=3)
                        if nch > 0:
                            nc.vector.tensor_add(u1[:pt, :], pa[:pt, :], st[:pt, :])
                        else:
                            nc.vector.tensor_copy(out=u1[:pt, :], in_=st[:pt, :])
                        u2 = pool.tile([P, D_HID], f32, tag="u2", bufs=3)
                        nc.vector.tensor_mul(u2[:pt, :], u1[:pt, :], w2_t[:pt, :])
                        yv = pool.tile([P, 1], f32, tag="yv", bufs=3)
                        nc.vector.tensor_reduce(
                            yv[:pt, :],
                            u2[:pt, :],
                            axis=mybir.AxisListType.X,
                            op=mybir.AluOpType.add,
                        )
                        ov = pool.tile([P, 1], f32, tag="ov", bufs=3)
                        nc.scalar.activation(
                            ov[:pt, :],
                            yv[:pt, :],
                            mybir.ActivationFunctionType.Sigmoid,
                            bias=b2_t[:pt, :],
                            scale=dv,
                        )
                        nc.scalar.dma_start(y_out[t * P : t * P + pt, :], ov[:pt, :])

            # ---- phase C: layer 1 (table = h_full, self rows = local h_sh)
            agg_layer(h_full, h_sh, layer=1)

            # ---- phase D: AllGather R'
            nc.gpsimd.collective_compute(
                "AllGather",
                mybir.AluOpType.bypass,
                replica_groups=rg,
                ins=[r_sh[:]],
                outs=[r_full[:]],
            )

            # ---- phase E: layer 2
            agg_layer(r_full, r_sh, layer=2)

    nc.compile()
    return nc


# ---------------------------------------------------------------------------


def kernel(**inputs) -> np.ndarray:
    global LAST_RESULTS
    x = np.asarray(inputs["x"], dtype=np.float32)
    edge_index = np.asarray(inputs["edge_index"])
    w1_in = np.asarray(inputs["W1"], dtype=np.float32)
    b1_in = np.asarray(inputs["b1"], dtype=np.float32)
    w2_in = np.asarray(inputs["W2"], dtype=np.float32)
    b2_in = np.asarray(inputs["b2"], dtype=np.float32)

    n_nodes = x.shape[0]
    src = edge_index[0].astype(np.int64)
    dst = edge_index[1].astype(np.int64)

    deg = np.bincount(dst, minlength=n_nodes).astype(np.float64) + 1.0
    dinv = (1.0 / np.sqrt(deg)).astype(np.float32)

    meta, per_core = _build_plan(src, dst, n_nodes, N_CORES)
    sh = meta["sh"]
    n_tiles = meta["n_tiles"]

    nc = _build_program(meta)

    iota_arr = np.broadcast_to(np.arange(P, dtype=np.float32), (P, P)).copy()
    iota4_arr = np.broadcast_to(
        np.arange(P, dtype=np.float32), (P, 4, P)
    ).copy()
    b1bc = np.broadcast_to(b1_in.reshape(1, D_HID), (P, D_HID)).copy()
    w2bc = np.broadcast_to(w2_in.reshape(1, D_HID), (P, D_HID)).copy()

    in_maps = []
    for c in range(N_CORES):
        idx_arr, dst_arr = per_core[c]
        xs = x[c * sh : (c + 1) * sh]  # [sh, 128]
        xT = np.ascontiguousarray(xs.T)  # [128, sh]
        dv = np.zeros((P, n_tiles), dtype=np.float32)
        dsl = dinv[c * sh : (c + 1) * sh]
        for t in range(n_tiles):
            pt = min(P, sh - t * P)
            dv[:pt, t] = dsl[t * P : t * P + pt]
        in_maps.append(
            {
                "xT": xT,
                "w1": w1_in,
                "b1bc": b1bc,
                "w2bc": w2bc,
                "dinv_sh": dv,
                "iota": iota_arr,
                "iota4": iota4_arr,
                "idx16": idx_arr,
                "dstloc": dst_arr,
                "b2col": np.full((P, 1), float(b2_in.reshape(-1)[0]), dtype=np.float32),
            }
        )

    from concourse import bass_utils

    if os.environ.get("BASS_TRACE"):
        _install_axon_profile_shim()

    res = bass_utils.run_bass_kernel_spmd(
        nc,
        in_maps,
        core_ids=list(range(N_CORES)),
        trace=bool(os.environ.get("BASS_TRACE")),
        trace_cores=[0] if os.environ.get("BASS_TRACE") else None,
    )
    LAST_RESULTS = res
    out = np.concatenate([res.results[c]["y"] for c in range(N_CORES)], axis=0)
    return out.astype(np.float32)


# revision 7
# speedup vs baseline: 1.4385x; 1.0209x over previous
"""CreditRiskGNN (2-layer GCN) Trainium2 kernel, 8 NeuronCores.

Sharding (per spec hint): nodes sharded across the 8 cores; edges partitioned
by destination node so scatter-adds are core-local; the per-shard node
features are all-gathered between layers.

Math: GCNConv(x, W, b)[d] = dinv[d] * (sum_{e: dst=d} h'[src_e] + h'[d]) + b
where h' = dinv (.) (x @ W) and dinv = rsqrt(indegree + 1) (self-loops).
Layer 2 uses (A @ R') @ W2 == A @ (R' @ W2) associativity so both layers share
one aggregation structure.

Device pipeline per core (one NEFF, SPMD on all 8 cores; per-core data only):
  A) h'_shard = dinv (.) (x_shard @ W1)        (PE matmul + DVE scale)
  B) AllGather h' -> full table [N, 64] in DRAM
  C) layer-1 aggregation per 128-dst tile: dma_gather of h'[src] rows
     (SWDGE ucode, 4 queues round-robin), one-hot dst-selection built on DVE
     (is_equal vs iota), PE matmuls accumulate into PSUM; fused epilogue
     R' = dinv (.) relu(dinv (.) (agg + self) + b1)
  D) AllGather R'
  E) layer-2 aggregation over the same edges; y = sigmoid(dinv*(agg2@W2)+b2)

Host does graph preprocessing only (CSR sharding, degree counts, gather-index
layout) and the final shard concat.
"""

import contextlib
import ctypes
import math
import os
import sys
import types

import ml_dtypes
import numpy as np

N_CORES = 8
P = 128
D_HID = 64
WIN = 32768                # int16 index window for dma_gather
MAX_IDX_PER_GATHER = 1024  # HW descriptor-ring limit (2048 hangs the queue)

LAST_RESULTS = None  # BassKernelResults of the last run (for test harnesses)


# ---------------------------------------------------------------------------
# axon NTFF profile hook shim (only needed when BASS_TRACE=1 under axon)
def _install_axon_profile_shim():
    if "antenv.axon_hooks" in sys.modules:
        return
    try:
        so_path = "/opt/axon/libaxon_pjrt.so"
        if not os.path.exists(so_path):
            return
        lib = ctypes.CDLL(so_path)
        if not hasattr(lib, "axon_start_nrt_profile"):
            return
        lib.axon_start_nrt_profile.argtypes = [
            ctypes.POINTER(ctypes.c_int64),
            ctypes.c_size_t,
        ]
        lib.axon_start_nrt_profile.restype = ctypes.c_int64
        lib.axon_stop_nrt_profile.argtypes = [ctypes.c_char_p]
        lib.axon_stop_nrt_profile.restype = ctypes.c_int64

        @contextlib.contextmanager
        def _hook(output_dir, device_ids):
            import jax

            jax.devices()
            if device_ids:
                ids = (ctypes.c_int64 * len(device_ids))(*device_ids)
                rc = lib.axon_start_nrt_profile(ids, len(device_ids))
            else:
                rc = lib.axon_start_nrt_profile(None, 0)
            if rc != 0:
                raise RuntimeError(f"axon_start_nrt_profile rc={rc}")
            try:
                yield
            finally:
                n = lib.axon_stop_nrt_profile(str(output_dir).encode())
                if n < 0:
                    raise RuntimeError(f"axon_stop_nrt_profile rc={n}")

        mod = types.ModuleType("antenv.axon_hooks")
        _state = {"hook": _hook}
        mod.set_axon_ntff_profile_hook = lambda h: _state.__setitem__("hook", h)
        mod.get_axon_ntff_profile_hook = lambda: _state["hook"]
        sys.modules["antenv.axon_hooks"] = mod
        import antenv

        antenv.axon_hooks = mod
    except Exception:
        pass


# ---------------------------------------------------------------------------
# Host-side graph preprocessing


def _wrap_idx_block(idxs_i16: np.ndarray) -> np.ndarray:
    """[n] int16 -> [128, n//16] in the SWDGE ucode layout: idx i at
    [i%16, i//16], replicated across the 8 groups of 16 partitions."""
    n = idxs_i16.shape[0]
    block = np.zeros((16, n // 16), dtype=np.int16)
    i = np.arange(n)
    block[i % 16, i // 16] = idxs_i16
    return np.tile(block, (8, 1))


def _build_plan(src, dst, n_nodes, n_cores):
    """Partition edges by destination shard; group per (dst-tile, src-window);
    pad each group to the max count across cores (rounded to 128) so the
    program shape is identical on every core."""
    sh = n_nodes // n_cores
    n_tiles = math.ceil(sh / P)
    n_win = math.ceil(n_nodes / WIN)

    core_of = dst // sh
    counts = np.zeros((n_cores, n_tiles, n_win), dtype=np.int64)
    per_core_sorted = []
    for c in range(n_cores):
        m = core_of == c
        s_c = src[m].astype(np.int64)
        d_c = (dst[m] - c * sh).astype(np.int64)
        tile_id = d_c // P
        win = s_c // WIN
        order = np.lexsort((s_c, win, tile_id))
        s_c, d_c = s_c[order], d_c[order]
        key = (d_c // P) * n_win + (s_c // WIN)
        allkeys = np.arange(n_tiles * n_win)
        starts = np.searchsorted(key, allkeys, side="left").reshape(n_tiles, n_win)
        ends = np.searchsorted(key, allkeys, side="right").reshape(n_tiles, n_win)
        counts[c] = ends - starts
        per_core_sorted.append((s_c, d_c, starts))

    padded = counts.max(axis=0)
    padded = np.where(padded > 0, ((padded + 15) // 16) * 16, 0).astype(np.int64)

    groups = []  # (t, w, ni, idx_off16, chunk_off)
    tile_nch = np.zeros(n_tiles, dtype=np.int64)
    tile_chunk_off = np.zeros(n_tiles, dtype=np.int64)
    off16 = 0
    chunk_off = 0
    for t in range(n_tiles):
        tile_chunk_off[t] = chunk_off
        for w in range(n_win):
            ni = int(padded[t, w])
            if ni == 0:
                continue
            groups.append((t, w, ni, off16, chunk_off))
            off16 += ni // 16
            chunk_off += (ni + P - 1) // P
        tile_nch[t] = chunk_off - tile_chunk_off[t]

    chunk_k = np.zeros(int(chunk_off), dtype=np.int64)
    for (t, w, ni, o16, ch_off) in groups:
        ncg = (ni + P - 1) // P
        for k in range(ncg):
            chunk_k[ch_off + k] = min(P, ni - k * P)

    meta = dict(
        n_nodes=n_nodes,
        sh=sh,
        n_tiles=n_tiles,
        n_win=n_win,
        groups=groups,
        tile_nch=tile_nch,
        tile_chunk_off=tile_chunk_off,
        chunk_k=chunk_k,
        total_idx=off16 * 16,
        total_chunks=int(chunk_off),
    )

    per_core = []
    for c in range(n_cores):
        s_c, d_c, starts = per_core_sorted[c]
        idx_arr = np.zeros((P, meta["total_idx"] // 16), dtype=np.int16)
        dst_arr = np.full((P, meta["total_chunks"]), -1.0, dtype=np.float32)
        for (t, w, ni, o16, ch_off) in groups:
            n_real = int(counts[c, t, w])
            st = int(starts[t, w])
            rel = np.zeros(ni, dtype=np.int16)
            if n_real > 0:
                rel[:n_real] = (s_c[st : st + n_real] - w * WIN).astype(np.int16)
            idx_arr[:, o16 : o16 + ni // 16] = _wrap_idx_block(rel)
            if n_real > 0:
                i = np.arange(n_real)
                dst_arr[i % P, ch_off + i // P] = (d_c[st : st + n_real] % P).astype(
                    np.float32
                )
        per_core.append((idx_arr, dst_arr))
    return meta, per_core


# ---------------------------------------------------------------------------
# Device program


def _build_program(meta):
    import concourse.bacc as bacc
    import concourse.mybir as mybir
    import concourse.tile as tile

    n_nodes = meta["n_nodes"]
    sh = meta["sh"]
    n_tiles = meta["n_tiles"]
    groups = meta["groups"]
    tile_nch = meta["tile_nch"]
    tile_chunk_off = meta["tile_chunk_off"]
    chunk_k = meta["chunk_k"]
    total_idx = meta["total_idx"]
    total_chunks = meta["total_chunks"]

    f32 = mybir.dt.float32
    nc = bacc.Bacc("TRN2", target_bir_lowering=False, debug=False, num_swdge_queues=4)

    xT = nc.dram_tensor("xT", [P, sh], f32, kind="ExternalInput")
    w1 = nc.dram_tensor("w1", [P, D_HID], f32, kind="ExternalInput")
    b1bc = nc.dram_tensor("b1bc", [P, D_HID], f32, kind="ExternalInput")
    w2bc = nc.dram_tensor("w2bc", [P, D_HID], f32, kind="ExternalInput")
    dinv_sh = nc.dram_tensor("dinv_sh", [P, n_tiles], f32, kind="ExternalInput")
    iota = nc.dram_tensor("iota", [P, P], f32, kind="ExternalInput")
    iota4 = nc.dram_tensor("iota4", [P, 4, P], mybir.dt.bfloat16, kind="ExternalInput")
    idx16 = nc.dram_tensor(
        "idx16", [P, total_idx // 16], mybir.dt.int16, kind="ExternalInput"
    )
    dstloc = nc.dram_tensor("dstloc", [P, total_chunks], mybir.dt.bfloat16, kind="ExternalInput")
    b2col = nc.dram_tensor("b2col", [P, 1], f32, kind="ExternalInput")
    y_out = nc.dram_tensor("y", [sh, 1], f32, kind="ExternalOutput")

    h_sh = nc.dram_tensor("h_sh", [sh, D_HID], f32, kind="Internal")
    h_full = nc.dram_tensor(
        "h_full", [n_nodes, D_HID], f32, kind="Internal", addr_space="Shared"
    )
    r_sh = nc.dram_tensor("r_sh", [sh, D_HID], f32, kind="Internal")
    r_full = nc.dram_tensor(
        "r_full", [n_nodes, D_HID], f32, kind="Internal", addr_space="Shared"
    )

    rg = [list(range(N_CORES))]

    with tile.TileContext(nc) as tc:
        with (
            tc.tile_pool(name="const", bufs=1) as cpool,
            tc.tile_pool(name="sbuf", bufs=1) as pool,
            tc.tile_pool(name="psum", bufs=1, space="PSUM") as psum_pool,
        ):
            w1_t = cpool.tile([P, D_HID], f32)
            nc.sync.dma_start(w1_t[:], w1[:])
            b1_t = cpool.tile([P, D_HID], f32)
            nc.sync.dma_start(b1_t[:], b1bc[:])
            w2_t = cpool.tile([P, D_HID], f32)
            nc.sync.dma_start(w2_t[:], w2bc[:])
            dinv_t = cpool.tile([P, n_tiles], f32)
            nc.sync.dma_start(dinv_t[:], dinv_sh[:])
            iota_t = cpool.tile([P, P], f32)
            nc.sync.dma_start(iota_t[:], iota[:])
            iota4_t = cpool.tile([P, 4, P], mybir.dt.bfloat16)
            nc.sync.dma_start(iota4_t[:], iota4[:])
            idx_t = cpool.tile([P, total_idx // 16], mybir.dt.int16)
            nc.sync.dma_start(idx_t[:], idx16[:])
            dl_t = cpool.tile([P, total_chunks], mybir.dt.bfloat16)
            nc.sync.dma_start(dl_t[:], dstloc[:])
            b2_t = cpool.tile([P, 1], f32)
            nc.sync.dma_start(b2_t[:], b2col[:])

            # ---- phase A: h' = dinv (.) (x @ W1) -> h_sh
            for t in range(n_tiles):
                pt = min(P, sh - t * P)
                xt = pool.tile([P, P], f32, tag="xt", bufs=3)
                nc.sync.dma_start(xt[:, :pt], xT[:, t * P : t * P + pt])
                ph = psum_pool.tile([P, D_HID], f32, tag="ph", bufs=2, space="PSUM")
                nc.tensor.matmul(
                    ph[:pt, :], lhsT=xt[:, :pt], rhs=w1_t[:], start=True, stop=True
                )
                hs = pool.tile([P, D_HID], f32, tag="hs", bufs=3)
                nc.vector.tensor_scalar_mul(
                    hs[:pt, :], ph[:pt, :], dinv_t[:pt, t : t + 1]
                )
                nc.sync.dma_start(h_sh[t * P : t * P + pt, :], hs[:pt, :])

            # ---- phase B: AllGather h'
            nc.gpsimd.collective_compute(
                "AllGather",
                mybir.AluOpType.bypass,
                replica_groups=rg,
                ins=[h_sh[:]],
                outs=[h_full[:]],
            )

            qn_state = [0]

            def agg_layer(table, self_src, layer):
                for t in range(n_tiles):
                    pt = min(P, sh - t * P)
                    nch = int(tile_nch[t])
                    ch0 = int(tile_chunk_off[t])
                    if nch > 0:
                        gbuf = pool.tile(
                            [P, nch, D_HID], f32, tag=f"g{layer}", bufs=3
                        )
                        gbf = pool.tile(
                            [P, nch, D_HID], mybir.dt.bfloat16, tag=f"gb{layer}", bufs=3
                        )
                        col = 0
                        for (gt, w, ni, o16, ch_off) in groups:
                            if gt != t:
                                continue
                            base = w * WIN
                            span = min(WIN, n_nodes - base)
                            done = 0
                            while done < ni:
                                take = min(MAX_IDX_PER_GATHER, ni - done)
                                ncg = (take + P - 1) // P
                                nc.gpsimd.dma_gather(
                                    gbuf[:, col : col + ncg, :],
                                    table[base : base + span, :],
                                    idx_t[
                                        :,
                                        o16 + done // 16 : o16 + (done + take) // 16,
                                    ],
                                    take,
                                    take,
                                    D_HID,
                                    queue_num=qn_state[0] % 4,
                                )
                                qn_state[0] += 1
                                done += take
                                col += ncg
                    if nch > 0:
                        nc.vector.tensor_copy(out=gbf[:], in_=gbuf[:])
                    st = pool.tile([P, D_HID], f32, tag=f"st{layer}", bufs=3)
                    if pt < P:
                        nc.vector.memset(st[:], 0.0)
                    nc.sync.dma_start(st[:pt, :], self_src[t * P : t * P + pt, :])
                    if nch > 0:
                        pa = psum_pool.tile(
                            [P, D_HID], f32, tag=f"pa{layer}", bufs=2, space="PSUM"
                        )
                        for cb in range(0, nch, 4):
                            b = min(4, nch - cb)
                            oh = pool.tile([P, 4, P], mybir.dt.bfloat16, tag=f"oh{layer}", bufs=4)
                            dls = dl_t[:, ch0 + cb : ch0 + cb + b].rearrange(
                                "p (b o) -> p b o", o=1
                            )
                            nc.vector.tensor_tensor(
                                out=oh[:, :b, :],
                                in0=dls.to_broadcast([P, b, P]),
                                in1=iota4_t[:, :b, :],
                                op=mybir.AluOpType.is_equal,
                            )
                            for k in range(b):
                                ch = cb + k
                                kk = int(chunk_k[ch0 + ch])
                                nc.tensor.matmul(
                                    pa[:],
                                    lhsT=oh[:kk, k, :],
                                    rhs=gbf[:kk, ch, :],
                                    start=(ch == 0),
                                    stop=(ch == nch - 1),
                                )
                    dv = dinv_t[:pt, t : t + 1]
                    if layer == 1:
                        t1 = pool.tile([P, D_HID], f32, tag="t1", bufs=3)
                        if nch > 0:
                            nc.vector.tensor_add(t1[:pt, :], pa[:pt, :], st[:pt, :])
                        else:
                            nc.vector.tensor_copy(out=t1[:pt, :], in_=st[:pt, :])
                        t2 = pool.tile([P, D_HID], f32, tag="t2", bufs=3)
                        nc.vector.tensor_scalar_mul(t2[:pt, :], t1[:pt, :], dv)
                        t3 = pool.tile([P, D_HID], f32, tag="t3", bufs=3)
                        nc.vector.tensor_add(t3[:pt, :], t2[:pt, :], b1_t[:pt, :])
                        rr = pool.tile([P, D_HID], f32, tag="rr", bufs=3)
                        nc.scalar.activation(
                            rr[:pt, :], t3[:pt, :], mybir.ActivationFunctionType.Relu
                        )
                        rp = pool.tile([P, D_HID], f32, tag="rp", bufs=3)
                        nc.vector.tensor_scalar_mul(rp[:pt, :], rr[:pt, :], dv)
                        nc.scalar.dma_start(r_sh[t * P : t * P + pt, :], rp[:pt, :])
                    else:
                        u1 = pool.tile([P, D_HID], f32, tag="u1", bufs=3)
                        if nch > 0:
                            nc.vector.tensor_add(u1[:pt, :], pa[:pt, :], st[:pt, :])
                        else:
                            nc.vector.tensor_copy(out=u1[:pt, :], in_=st[:pt, :])
                        u2 = pool.tile([P, D_HID], f32, tag="u2", bufs=3)
                        nc.vector.tensor_mul(u2[:pt, :], u1[:pt, :], w2_t[:pt, :])
                        yv = pool.tile([P, 1], f32, tag="yv", bufs=3)
                        nc.vector.tensor_reduce(
                            yv[:pt, :],
                            u2[:pt, :],
                            axis=mybir.AxisListType.X,
                            op=mybir.AluOpType.add,
                        )
                        ov = pool.tile([P, 1], f32, tag="ov", bufs=3)
                        nc.scalar.activation(
                            ov[:pt, :],
                            yv[:pt, :],
                            mybir.ActivationFunctionType.Sigmoid,
                            bias=b2_t[:pt, :],
                            scale=dv,
                        )
                        nc.scalar.dma_start(y_out[t * P : t * P + pt, :], ov[:pt, :])

            # ---- phase C: layer 1 (table = h_full, self rows = local h_sh)
            agg_layer(h_full, h_sh, layer=1)

            # ---- phase D: AllGather R'
            nc.gpsimd.collective_compute(
                "AllGather",
                mybir.AluOpType.bypass,
                replica_groups=rg,
                ins=[r_sh[:]],
                outs=[r_full[:]],
            )

            # ---- phase E: layer 2
            agg_layer(r_full, r_sh, layer=2)

    nc.compile()
    return nc


# ---------------------------------------------------------------------------


def kernel(**inputs) -> np.ndarray:
    global LAST_RESULTS
    x = np.asarray(inputs["x"], dtype=np.float32)
    edge_index = np.asarray(inputs["edge_index"])
    w1_in = np.asarray(inputs["W1"], dtype=np.float32)
    b1_in = np.asarray(inputs["b1"], dtype=np.float32)
    w2_in = np.asarray(inputs["W2"], dtype=np.float32)
    b2_in = np.asarray(inputs["b2"], dtype=np.float32)

    n_nodes = x.shape[0]
    src = edge_index[0].astype(np.int64)
    dst = edge_index[1].astype(np.int64)

    deg = np.bincount(dst, minlength=n_nodes).astype(np.float64) + 1.0
    dinv = (1.0 / np.sqrt(deg)).astype(np.float32)

    meta, per_core = _build_plan(src, dst, n_nodes, N_CORES)
    sh = meta["sh"]
    n_tiles = meta["n_tiles"]

    nc = _build_program(meta)

    iota_arr = np.broadcast_to(np.arange(P, dtype=np.float32), (P, P)).copy()
    iota4_arr = (
        np.broadcast_to(np.arange(P, dtype=np.float32), (P, 4, P))
        .astype(ml_dtypes.bfloat16)
        .copy()
    )
    b1bc = np.broadcast_to(b1_in.reshape(1, D_HID), (P, D_HID)).copy()
    w2bc = np.broadcast_to(w2_in.reshape(1, D_HID), (P, D_HID)).copy()

    in_maps = []
    for c in range(N_CORES):
        idx_arr, dst_arr = per_core[c]
        xs = x[c * sh : (c + 1) * sh]  # [sh, 128]
        xT = np.ascontiguousarray(xs.T)  # [128, sh]
        dv = np.zeros((P, n_tiles), dtype=np.float32)
        dsl = dinv[c * sh : (c + 1) * sh]
        for t in range(n_tiles):
            pt = min(P, sh - t * P)
            dv[:pt, t] = dsl[t * P : t * P + pt]
        in_maps.append(
            {
                "xT": xT,
                "w1": w1_in,
                "b1bc": b1bc,
                "w2bc": w2bc,
                "dinv_sh": dv,
                "iota": iota_arr,
                "iota4": iota4_arr,
                "idx16": idx_arr,
                "dstloc": dst_arr.astype(ml_dtypes.bfloat16),
                "b2col": np.full((P, 1), float(b2_in.reshape(-1)[0]), dtype=np.float32),
            }
        )

    from concourse import bass_utils

    if os.environ.get("BASS_TRACE"):
        _install_axon_profile_shim()

    res = bass_utils.run_bass_kernel_spmd(
        nc,
        in_maps,
        core_ids=list(range(N_CORES)),
        trace=bool(os.environ.get("BASS_TRACE")),
        trace_cores=[0] if os.environ.get("BASS_TRACE") else None,
    )
    LAST_RESULTS = res
    out = np.concatenate([res.results[c]["y"] for c in range(N_CORES)], axis=0)
    return out.astype(np.float32)
